# revision 7
# baseline (speedup 1.0000x reference)
"""Trainium2 Bass kernel for DifferentiableSoftmaxTree NLL (hierarchical
softmax negative log-likelihood).

Math: the 2-way log_softmax at each tree node reduces to a softplus of a
signed logit difference: for sample b with path nodes n_k / directions d_k,
    s_k  = features[b] . (node_weights[n_k,:,1] - node_weights[n_k,:,0])
    out[b] = sum_k mask_k * softplus((1-2 d_k) * s_k)

v2 strategy (data-parallel over batch, 8 cores x 512 samples), exploiting
that the top tree levels are SHARED across samples:

  * Levels 0..8 (nodes 0..510, on every path): computed DENSELY on the
    TensorEngine. Per 128-sample block, 4 accumulating bf16 matmuls
    (contraction over feature chunks of 128) produce U[sample, node] for
    all 511 top nodes in PSUM. Per-level ``tensor_tensor_reduce`` against
    a host-built +-1 one-hot (sign = direction) selects each sample's
    signed logit. Replaces 9/16 of the per-sample gather traffic with a
    1KB/sample mask.
  * Levels 9..15 (near-distinct nodes): per-class contiguous bf16 gather
    of sgn*mask-folded diff rows (3.5KB/sample vs the old 32KB), one
    single-offset SWDGE indirect DMA per 128-sample block, then 7 fused
    multiply-reduce ``tensor_tensor_reduce`` ops against the feature row.
  * Selection/reduction: elementwise product (DVE tensor_tensor, bf16 2x
    mode) followed by per-level tensor_reduce on DVE, with 2 of the 7 deep
    levels reduced on ACT (Identity + accum_out) for engine balance.
    (tensor_tensor_reduce would fuse these but wedges this runtime.)
  * Missing level 15 (classes with 15-edge paths) is folded into a
    -30 bias column added to sel: softplus(-30) ~= 0.
  * softplus(x) = ln(1 + exp(x)) as two ACT ops (|sel| << 88 so exp can't
    overflow); Exp/Ln/Identity share one ACT table
    (natural_log_exp_and_others) so there is a single table load, and
    accum_out on the Ln op yields the per-sample sum directly.

All tables/masks are bf16 (rel err ~1.5e-3 vs the 2e-2 gate, validated in
numpy). Column layouts keep every DVE operand 4-byte aligned so it runs
in 2x packed mode (node n lives at column n+1; column 0 is zero).
"""

import numpy as np
from contextlib import ExitStack

import concourse.bass as bass
import concourse.mybir as mybir
import concourse.tile as tile
from concourse import bass_utils
import concourse.bacc as bacc
import ml_dtypes

NUM_CLASSES = 50000
NUM_INTERNAL = NUM_CLASSES - 1
D = 512
B = 4096
K = 16
TOPL = 9                   # levels via dense matmul
DEEPL = K - TOPL           # 7 levels via gather
NTOP = 2 ** TOPL - 1       # 511 nodes in levels 0..8
NTOPP = NTOP + 1           # padded: node n -> col n+1, col 0 = 0
N_CORES = 8
BL = B // N_CORES          # samples per core
P = 128                    # partition dim
NBLK = BL // P             # 128-sample blocks per core
NCH = D // P               # feature chunks for matmul contraction

BF16 = ml_dtypes.bfloat16

_AF = mybir.ActivationFunctionType
_OP = mybir.AluOpType


def _build_program():
    nc = bacc.Bacc(
        "TRN2",
        target_bir_lowering=False,
        debug=False,
        enable_asserts=False,
        num_devices=N_CORES,
    )
    # per-core inputs
    ptabd_ap = nc.dram_tensor(
        "ptabd", [NUM_CLASSES, DEEPL * D], mybir.dt.bfloat16, kind="ExternalInput"
    ).ap()
    idx_ap = nc.dram_tensor("idx", [P, NBLK], mybir.dt.int32, kind="ExternalInput").ap()
    featb_ap = nc.dram_tensor(
        "featb", [P, NBLK * D], mybir.dt.bfloat16, kind="ExternalInput"
    ).ap()
    featT_ap = nc.dram_tensor(
        "featT", [P, NCH * BL], mybir.dt.bfloat16, kind="ExternalInput"
    ).ap()
    dtopT_ap = nc.dram_tensor(
        "dtopT", [P, NCH * NTOPP], mybir.dt.bfloat16, kind="ExternalInput"
    ).ap()
    meta_ap = nc.dram_tensor(
        "meta", [BL, NTOPP + K], mybir.dt.bfloat16, kind="ExternalInput"
    ).ap()
    out_ap = nc.dram_tensor("out", [BL, 1], mybir.dt.float32, kind="ExternalOutput").ap()

    with tile.TileContext(nc) as tc, ExitStack() as ctx:
        once_pool = ctx.enter_context(tc.tile_pool(name="once", bufs=1))
        meta_pool = ctx.enter_context(tc.tile_pool(name="meta", bufs=2))
        gath_pool = ctx.enter_context(tc.tile_pool(name="gath", bufs=3))
        ub_pool = ctx.enter_context(tc.tile_pool(name="ub", bufs=2))
        scr_pool = ctx.enter_context(tc.tile_pool(name="scr", bufs=2))
        sel_pool = ctx.enter_context(tc.tile_pool(name="sel", bufs=2))
        psum_pool = ctx.enter_context(tc.tile_pool(name="psum", bufs=2, space="PSUM"))

        idx_t = once_pool.tile([P, NBLK], mybir.dt.int32, tag="idx")
        nc.sync.dma_start(idx_t[:], idx_ap[:])
        featb_t = once_pool.tile([P, NBLK * D], mybir.dt.bfloat16, tag="featb")
        nc.sync.dma_start(featb_t[:], featb_ap[:])
        featT_t = once_pool.tile([P, NCH * BL], mybir.dt.bfloat16, tag="featT")
        nc.sync.dma_start(featT_t[:], featT_ap[:])
        dtopT_t = once_pool.tile([P, NCH * NTOPP], mybir.dt.bfloat16, tag="dtopT")
        nc.sync.dma_start(dtopT_t[:], dtopT_ap[:])

        for blk in range(NBLK):
            b0 = blk * P
            meta_t = meta_pool.tile([P, NTOPP + K], mybir.dt.bfloat16, tag="meta")
            nc.sync.dma_start(meta_t[:], meta_ap[b0 : b0 + P, :])

            # deep-path gather: one contiguous 3.5KB row per sample
            g_t = gath_pool.tile([P, DEEPL * D], mybir.dt.bfloat16, tag="g")
            nc.gpsimd.indirect_dma_start(
                out=g_t[:],
                out_offset=None,
                in_=ptabd_ap[:],
                in_offset=bass.IndirectOffsetOnAxis(ap=idx_t[:, blk : blk + 1], axis=0),
            )

            # top levels: U[sample, col] = sum_d feat[sample, d]*difftop[col-1, d]
            u_t = psum_pool.tile([P, NTOPP], mybir.dt.float32, tag="u")
            for c in range(NCH):
                nc.tensor.matmul(
                    u_t[:],
                    lhsT=featT_t[:, c * BL + b0 : c * BL + b0 + P],
                    rhs=dtopT_t[:, c * NTOPP : (c + 1) * NTOPP],
                    start=(c == 0),
                    stop=(c == NCH - 1),
                )
            ub_t = ub_pool.tile([P, NTOPP], mybir.dt.bfloat16, tag="ub")
            nc.scalar.activation(ub_t[:], u_t[:], _AF.Identity)

            sel_t = sel_pool.tile([P, K], mybir.dt.float32, tag="sel")
            # top selection: prodT = U * onehot(+-1), then per-level reduce
            prT_t = ub_pool.tile([P, NTOPP], mybir.dt.bfloat16, tag="prT")
            nc.vector.tensor_tensor(
                out=prT_t[:], in0=ub_t[:], in1=meta_t[:, 0:NTOPP], op=_OP.mult
            )
            for j in range(TOPL):
                lo, hi = 2 ** j, 2 ** (j + 1)
                nc.vector.tensor_reduce(
                    out=sel_t[:, j : j + 1],
                    in_=prT_t[:, lo:hi],
                    axis=mybir.AxisListType.X,
                    op=_OP.add,
                )
            # deep levels: prod = g * feat row, reduce 5 levels on DVE + 2 on ACT
            pr_t = scr_pool.tile([P, DEEPL * D], mybir.dt.bfloat16, tag="pr")
            nc.vector.tensor_tensor(
                out=pr_t[:].rearrange("p (k d) -> p k d", k=DEEPL),
                in0=g_t[:].rearrange("p (k d) -> p k d", k=DEEPL),
                in1=featb_t[:, blk * D : (blk + 1) * D][:, None, :].to_broadcast(
                    [P, DEEPL, D]
                ),
                op=_OP.mult,
            )
            RDVE = 5
            nc.vector.tensor_reduce(
                out=sel_t[:, TOPL : TOPL + RDVE],
                in_=pr_t[:, 0 : RDVE * D].rearrange("p (k d) -> p k d", k=RDVE),
                axis=mybir.AxisListType.X,
                op=_OP.add,
            )
            dump_t = scr_pool.tile([P, D], mybir.dt.bfloat16, tag="dump")
            for i in range(RDVE, DEEPL):
                nc.scalar.activation(
                    dump_t[:],
                    pr_t[:, i * D : (i + 1) * D],
                    _AF.Identity,
                    accum_out=sel_t[:, TOPL + i : TOPL + i + 1],
                )
            # masked-level-15 bias: sel += biasK (0 or -30)
            nc.vector.tensor_tensor(
                out=sel_t[:],
                in0=sel_t[:],
                in1=meta_t[:, NTOPP : NTOPP + K],
                op=_OP.add,
            )

            # out[b] = sum_k softplus(sel[b,k]) = sum_k ln(1 + exp(sel[b,k])).
            # |sel| <= ~50 so exp can't overflow fp32; Exp/Ln/Identity share
            # one ACT table (natural_log_exp_and_others). accum_out on the Ln
            # op produces the per-sample sum directly.
            e_t = sel_pool.tile([P, K], mybir.dt.float32, tag="e")
            nc.scalar.activation(e_t[:], sel_t[:], _AF.Exp)
            sp_t = sel_pool.tile([P, K], mybir.dt.float32, tag="sp")
            res_t = sel_pool.tile([P, 1], mybir.dt.float32, tag="res")
            nc.scalar.activation(
                sp_t[:], e_t[:], _AF.Ln, bias=1.0, accum_out=res_t[:]
            )
            nc.sync.dma_start(out_ap[b0 : b0 + P, :], res_t[:])

    nc.compile()
    return nc


_PROGRAM_CACHE = {}


def _get_program():
    if "nc" not in _PROGRAM_CACHE:
        _PROGRAM_CACHE["nc"] = _build_program()
    return _PROGRAM_CACHE["nc"]


def _reset_device():
    # A previously-crashed kernel can leave an exec unit wedged; a
    # client-side axon reset clears it and is near-free otherwise.
    try:
        import ctypes

        lib = ctypes.CDLL("/opt/axon/libaxon_pjrt.so")
        lib.axon_reset.restype = ctypes.c_int64
        lib.axon_reset()
    except Exception:
        pass


def _prepare_inputs(features, targets, node_weights, path_nodes_map, path_directions_map):
    feat = np.asarray(features, dtype=np.float32)
    t = np.asarray(targets, dtype=np.int32).reshape(-1)
    nw = np.asarray(node_weights, dtype=np.float32)
    pn = np.asarray(path_nodes_map, dtype=np.int32)
    pd = np.asarray(path_directions_map, dtype=np.int32)

    diff = nw[:, :, 1] - nw[:, :, 0]                        # [N_INT, D]

    # deep per-class table, levels 9..15, sign+mask folded, bf16
    nodes_d = pn[:, TOPL:]
    dirs_d = pd[:, TOPL:]
    maskd = nodes_d != -1
    safed = np.where(maskd, nodes_d, 0)
    sgnd = np.where(maskd, 1 - 2 * dirs_d, 0).astype(np.float32)
    ptabd = (diff[safed] * sgnd[:, :, None]).reshape(NUM_CLASSES, DEEPL * D)
    ptabd = np.ascontiguousarray(ptabd.astype(BF16))

    # top table transposed: dtopT[p, c*NTOPP + n+1] = diff[n, c*128+p]
    dtopT = np.zeros((P, NCH, NTOPP), np.float32)
    dtopT[:, :, 1:] = diff[:NTOP].reshape(NTOP, NCH, P).transpose(2, 1, 0)
    dtopT = np.ascontiguousarray(dtopT.reshape(P, NCH * NTOPP).astype(BF16))

    # per-sample meta: +-1 one-hot over padded top nodes | bias16
    n9 = pn[t, :TOPL]                                       # [B, 9] all valid
    d9 = pd[t, :TOPL]
    oh = np.zeros((B, NTOPP), np.float32)
    oh[np.arange(B)[:, None], n9 + 1] = 1 - 2 * d9
    biasK = np.zeros((B, K), np.float32)
    biasK[:, K - 1] = np.where(pn[t, K - 1] == -1, -30.0, 0.0)
    meta = np.ascontiguousarray(
        np.concatenate([oh, biasK], axis=1).astype(BF16)     # [B, NTOPP+K]
    )

    per_core = []
    for i in range(N_CORES):
        sl = slice(i * BL, (i + 1) * BL)
        fc = feat[sl]
        tc_ = t[sl]
        featb = np.ascontiguousarray(
            fc.reshape(NBLK, P, D).transpose(1, 0, 2).reshape(P, NBLK * D).astype(BF16)
        )
        featT = np.ascontiguousarray(
            fc.reshape(BL, NCH, P).transpose(2, 1, 0).reshape(P, NCH * BL).astype(BF16)
        )
        idx = np.ascontiguousarray(tc_.reshape(NBLK, P).T.astype(np.int32))
        per_core.append(
            {
                "ptabd": ptabd,
                "dtopT": dtopT,
                "idx": idx,
                "featb": featb,
                "featT": featT,
                "meta": meta[sl],
            }
        )
    return per_core


def kernel(features, targets, node_weights, path_nodes_map, path_directions_map):
    in_maps = _prepare_inputs(
        features, targets, node_weights, path_nodes_map, path_directions_map
    )
    _reset_device()
    nc = _get_program()
    res = bass_utils.run_bass_kernel_spmd(nc, in_maps, core_ids=list(range(N_CORES)))
    out = np.concatenate([res.results[i]["out"].reshape(-1) for i in range(N_CORES)])
    return out.astype(np.float32)
